# revision 3
# baseline (speedup 1.0000x reference)
"""AttnDecoderRNN single-step decoder on 8 TRN2 NeuronCores.

Strategy (tensor-parallel, per sharding hint):
 - out_w/out_b vocab-sharded 8 ways; each core streams its [2048, 6283] f32
   transposed shard from HBM and does the logits matvec on PE.
 - GRU weights row-sharded over hidden: core c computes h[c*128:(c+1)*128];
   h assembled with an AllGather.
 - Attention reassociated: scores = enc @ (h @ attn_w) + (attn_b . h); the
   attn_b term is constant across positions so it cancels in softmax and in
   every downstream output -> dropped. enc row-sharded over S (512 rows/core);
   local score shards AllGathered; softmax computed redundantly; per-shard
   context partials AllReduced.
 - Embedding: only row emb[word_input] is ever read; sliced host-side during
   input sharding and fed partition-major.
"""
import sys, os

for _p in ("/opt/trn_rl_repo", "/root/.axon_site/_ro/trn_rl_repo"):
    if os.path.isdir(_p) and _p not in sys.path:
        sys.path.append(_p)

import numpy as np
import concourse.bass as bass
import concourse.bacc as bacc
import concourse.mybir as mybir
import concourse.tile as tile
from concourse import bass_utils

f32 = mybir.dt.float32
AF = mybir.ActivationFunctionType
ALU = mybir.AluOpType
AX = mybir.AxisListType

NCORES = 8
H = 1024
HC = H // NCORES            # 128 hidden rows per core (GRU shard)
G = 3 * HC                  # 384 gate rows per core
S = 4096
SC = S // NCORES            # 512 encoder rows per core
V = 50257
VC = -(-V // NCORES)        # 6283 vocab rows per core
VPAD = VC * NCORES          # 50264
NV = 512                    # vocab tile width for the logits matvec
NVB = -(-VC // NV)          # 13 blocks (12x512 + 139)
NKC = 2 * H // 128          # 16 contraction chunks for the logits matvec
RG = [list(range(NCORES))]

_CACHE: dict = {}


def _build():
    nc = bacc.Bacc("TRN2", target_bir_lowering=False, debug=False,
                   num_devices=NCORES)

    def inp(name, shape):
        return nc.dram_tensor(name, shape, f32, kind="ExternalInput")

    x_pm_d = inp("x_pm", [128, 8])          # emb row, partition-major
    hp_pm_d = inp("hp_pm", [128, 8])        # prev hidden, partition-major
    hp_sl_d = inp("hp_sl", [1, HC])         # prev hidden, this core's slice
    wih_d = inp("wih_t", [H, G])            # w_ih[rows_c].T
    whh_d = inp("whh_t", [H, G])            # w_hh[rows_c].T
    brz_d = inp("b_rz", [1, 2 * HC])        # (b_ih+b_hh)[rows_c][:256]
    bin_d = inp("b_in", [1, HC])            # b_ih[rows_c][256:]
    bhn_d = inp("b_hn", [1, HC])            # b_hh[rows_c][256:]
    aw_d = inp("aw", [H, H])                # attn_w, natural layout
    enct_d = inp("enc_t", [H, SC])          # enc shard transposed
    encn_d = inp("enc_n", [SC, H])          # enc shard natural
    wt_d = inp("wt", [2 * H, VC])           # out_w shard transposed
    ob_d = inp("ob", [1, VC])               # out_b shard
    id_d = inp("ident", [128, 128])

    logits_d = nc.dram_tensor("logits", [1, VC], f32, kind="ExternalOutput")
    hidden_d = nc.dram_tensor("hidden", [1, H], f32, kind="ExternalOutput")
    attnw_d = nc.dram_tensor("attnw", [1, S], f32, kind="ExternalOutput")

    with tile.TileContext(nc) as tc:
        with (
            tc.tile_pool(name="cpool", bufs=1) as cpool,     # long-lived SBUF
            tc.tile_pool(name="vpool", bufs=1) as vpool,     # small vectors
            tc.tile_pool(name="spool", bufs=1) as spool,     # weight stream
            tc.tile_pool(name="ppool", bufs=1, space="PSUM") as ppool,
            tc.tile_pool(name="dpool", bufs=1, space="DRAM") as dpool,
        ):
            # ---------------- chain-critical input loads ----------------
            x_pm = cpool.tile([128, 8], f32)
            nc.sync.dma_start(x_pm[:], x_pm_d.ap())
            hp_pm = cpool.tile([128, 8], f32)
            nc.sync.dma_start(hp_pm[:], hp_pm_d.ap())
            hp_sl = cpool.tile([1, HC], f32)
            nc.sync.dma_start(hp_sl[:], hp_sl_d.ap())
            brz_t = cpool.tile([1, 2 * HC], f32)
            nc.sync.dma_start(brz_t[:], brz_d.ap())
            bin_t = cpool.tile([1, HC], f32)
            nc.sync.dma_start(bin_t[:], bin_d.ap())
            bhn_t = cpool.tile([1, HC], f32)
            nc.sync.dma_start(bhn_t[:], bhn_d.ap())
            ident = cpool.tile([128, 128], f32)
            nc.sync.dma_start(ident[:], id_d.ap())
            wih_t = cpool.tile([128, 8, G], f32)
            nc.sync.dma_start(wih_t[:], wih_d.ap().rearrange("(k p) g -> p k g", p=128))
            whh_t = cpool.tile([128, 8, G], f32)
            nc.sync.dma_start(whh_t[:], whh_d.ap().rearrange("(k p) g -> p k g", p=128))
            aw_t = cpool.tile([128, 8, H], f32)
            nc.sync.dma_start(aw_t[:], aw_d.ap().rearrange("(k p) j -> p k j", p=128))
            enct_t = cpool.tile([128, 8, SC], f32)
            nc.sync.dma_start(enct_t[:], enct_d.ap().rearrange("(k p) s -> p k s", p=128))
            encn_t = cpool.tile([128, 4, H], f32)
            nc.sync.dma_start(encn_t[:], encn_d.ap().rearrange("(i p) h -> p i h", p=128))

            # ---------------- GRU cell (rows_c shard) ----------------
            p_gi = ppool.tile([1, G], f32, tag="pv", bufs=3)
            for k in range(8):
                nc.tensor.matmul(p_gi[:], x_pm[:, k:k + 1], wih_t[:, k, :],
                                 start=(k == 0), stop=(k == 7))
            p_gh = ppool.tile([1, G], f32, tag="pv", bufs=3)
            for k in range(8):
                nc.tensor.matmul(p_gh[:], hp_pm[:, k:k + 1], whh_t[:, k, :],
                                 start=(k == 0), stop=(k == 7))

            gi_s = vpool.tile([1, G], f32)
            nc.scalar.copy(gi_s[:], p_gi[:])
            gsum = vpool.tile([1, 2 * HC], f32)
            nc.vector.tensor_add(gsum[:], gi_s[:, 0:2 * HC], p_gh[:, 0:2 * HC])
            nc.vector.tensor_add(gsum[:], gsum[:], brz_t[:])
            rz = vpool.tile([1, 2 * HC], f32)
            nc.scalar.activation(rz[:], gsum[:], AF.Sigmoid)
            ghn = vpool.tile([1, HC], f32)
            nc.vector.tensor_add(ghn[:], p_gh[:, 2 * HC:G], bhn_t[:])
            nc.vector.tensor_mul(ghn[:], rz[:, 0:HC], ghn[:])
            gin = vpool.tile([1, HC], f32)
            nc.vector.tensor_add(gin[:], gi_s[:, 2 * HC:G], bin_t[:])
            nc.vector.tensor_add(gin[:], gin[:], ghn[:])
            n_t = vpool.tile([1, HC], f32)
            nc.scalar.activation(n_t[:], gin[:], AF.Tanh)
            # h = n + z * (h_prev - n)
            d_t = vpool.tile([1, HC], f32)
            nc.vector.tensor_sub(d_t[:], hp_sl[:], n_t[:])
            nc.vector.tensor_mul(d_t[:], rz[:, HC:2 * HC], d_t[:])
            hc_t = vpool.tile([1, HC], f32)
            nc.vector.tensor_add(hc_t[:], n_t[:], d_t[:])

            # ---------------- AllGather h ----------------
            h_loc = dpool.tile([1, HC], f32)
            h_full = dpool.tile([1, H], f32)
            nc.sync.dma_start(h_loc[:], hc_t[:])
            nc.gpsimd.collective_compute(
                "AllGather", ALU.bypass, replica_groups=RG,
                ins=[h_loc.opt()], outs=[h_full.opt()])
            nc.sync.dma_start(hidden_d.ap(), h_full[:])
            h_pm = cpool.tile([128, 8], f32)
            nc.sync.dma_start(h_pm[:], h_full[:].rearrange("a (j p) -> p (a j)", p=128))

            # ---------------- u = h @ attn_w (partition-major out) ----------------
            p_u = ppool.tile([128, 8], f32, tag="pu", bufs=1)
            for j in range(8):
                for k in range(8):
                    nc.tensor.matmul(p_u[:, j:j + 1],
                                     aw_t[:, k, j * 128:(j + 1) * 128],
                                     h_pm[:, k:k + 1],
                                     start=(k == 0), stop=(k == 7))
            u_s = cpool.tile([128, 8], f32)
            nc.vector.tensor_copy(u_s[:], p_u[:])

            # ---------------- local scores = enc_c @ u ----------------
            p_sc = ppool.tile([1, SC], f32, tag="pv", bufs=3)
            for k in range(8):
                nc.tensor.matmul(p_sc[:], u_s[:, k:k + 1], enct_t[:, k, :],
                                 start=(k == 0), stop=(k == 7))
            sc_s = vpool.tile([1, SC], f32)
            nc.scalar.copy(sc_s[:], p_sc[:])

            # ---------------- AllGather scores ----------------
            sc_loc = dpool.tile([1, SC], f32)
            sc_full = dpool.tile([1, S], f32)
            nc.sync.dma_start(sc_loc[:], sc_s[:])
            nc.gpsimd.collective_compute(
                "AllGather", ALU.bypass, replica_groups=RG,
                ins=[sc_loc.opt()], outs=[sc_full.opt()])

            # ---------------- softmax (full, redundant on every core) ----------
            sp = cpool.tile([128, S // 128], f32)     # [128, 32] partition-major
            nc.sync.dma_start(sp[:], sc_full[:].rearrange("a (j p) -> p (a j)", p=128))
            m1 = vpool.tile([128, 1], f32)
            nc.vector.reduce_max(m1[:], sp[:], axis=AX.X)
            p_tr = ppool.tile([1, 128], f32, tag="pv", bufs=3)
            nc.tensor.transpose(p_tr[:], m1[:], ident[:])
            M_s = vpool.tile([1, 1], f32)
            nc.vector.reduce_max(M_s[:], p_tr[:], axis=AX.X)
            negM = vpool.tile([1, 1], f32)
            nc.vector.tensor_scalar_mul(negM[:], M_s[:], -1.0)
            ones_r = cpool.tile([1, 128], f32)
            nc.vector.memset(ones_r[:], 1.0)
            p_bc = ppool.tile([128, 1], f32, tag="pb", bufs=1)
            nc.tensor.matmul(p_bc[:], ones_r[:], negM[:], start=True, stop=True)
            negMb = cpool.tile([128, 1], f32)
            nc.vector.tensor_copy(negMb[:], p_bc[:])

            e_full = cpool.tile([128, S // 128], f32)
            z128 = vpool.tile([128, 1], f32)
            nc.scalar.activation(e_full[:], sp[:], AF.Exp,
                                 bias=negMb[:], accum_out=z128[:])
            p_tr2 = ppool.tile([1, 128], f32, tag="pv", bufs=3)
            nc.tensor.transpose(p_tr2[:], z128[:], ident[:])
            Z_s = vpool.tile([1, 1], f32)
            nc.vector.reduce_sum(Z_s[:], p_tr2[:], axis=AX.X)
            rZ = vpool.tile([1, 1], f32)
            nc.vector.reciprocal(rZ[:], Z_s[:])
            p_bc2 = ppool.tile([128, 1], f32, tag="pb", bufs=1)
            nc.tensor.matmul(p_bc2[:], ones_r[:], rZ[:], start=True, stop=True)
            rZb = cpool.tile([128, 1], f32)
            nc.vector.tensor_copy(rZb[:], p_bc2[:])

            w_full = cpool.tile([128, S // 128], f32)
            nc.vector.tensor_scalar_mul(w_full[:], e_full[:], rZb[:])
            nc.sync.dma_start(
                attnw_d.ap().rearrange("a (j p) -> p (a j)", p=128), w_full[:])

            # local softmax weights for the context partial (identical code on
            # every core; the data differs via sc_loc)
            sl_pm = cpool.tile([128, SC // 128], f32)   # [128, 4]
            nc.sync.dma_start(sl_pm[:], sc_loc[:].rearrange("a (j p) -> p (a j)", p=128))
            wl = cpool.tile([128, SC // 128], f32)
            nc.scalar.activation(wl[:], sl_pm[:], AF.Exp, bias=negMb[:])
            nc.vector.tensor_scalar_mul(wl[:], wl[:], rZb[:])

            # ---------------- context partial + AllReduce ----------------
            p_cx0 = ppool.tile([1, 512], f32, tag="pv", bufs=3)
            p_cx1 = ppool.tile([1, 512], f32, tag="pv", bufs=3)
            for i in range(4):
                nc.tensor.matmul(p_cx0[:], wl[:, i:i + 1], encn_t[:, i, 0:512],
                                 start=(i == 0), stop=(i == 3))
                nc.tensor.matmul(p_cx1[:], wl[:, i:i + 1], encn_t[:, i, 512:1024],
                                 start=(i == 0), stop=(i == 3))
            cx_s = vpool.tile([1, H], f32)
            nc.scalar.copy(cx_s[:, 0:512], p_cx0[:])
            nc.scalar.copy(cx_s[:, 512:1024], p_cx1[:])
            cx_loc = dpool.tile([1, H], f32)
            cx_full = dpool.tile([1, H], f32)
            nc.sync.dma_start(cx_loc[:], cx_s[:])
            nc.gpsimd.collective_compute(
                "AllReduce", ALU.add, replica_groups=RG,
                ins=[cx_loc.opt()], outs=[cx_full.opt()])

            # cat = [h ; context], partition-major [128, 16]
            cat_pm = cpool.tile([128, NKC], f32)
            nc.sync.dma_start(cat_pm[:, 0:8],
                              h_full[:].rearrange("a (j p) -> p (a j)", p=128))
            nc.sync.dma_start(cat_pm[:, 8:16],
                              cx_full[:].rearrange("a (j p) -> p (a j)", p=128))

            # ---------------- logits matvec: stream out_w^T shard ----------------
            for vb in range(NVB):
                off = vb * NV
                nv = min(NV, VC - off)
                wtile = spool.tile([128, NKC, NV], f32, tag="wt", bufs=2)
                nc.sync.dma_start(
                    wtile[:, :, 0:nv],
                    wt_d.ap()[:, off:off + nv].rearrange("(c p) n -> p c n", p=128))
                p_l = ppool.tile([1, NV], f32, tag="pl", bufs=3)
                for c in range(NKC):
                    nc.tensor.matmul(p_l[:, 0:nv], cat_pm[:, c:c + 1],
                                     wtile[:, c, 0:nv],
                                     start=(c == 0), stop=(c == NKC - 1))
                ob_t = spool.tile([1, NV], f32, tag="ob", bufs=2)
                nc.sync.dma_start(ob_t[:, 0:nv], ob_d.ap()[:, off:off + nv])
                lo_t = spool.tile([1, NV], f32, tag="lo", bufs=2)
                nc.vector.tensor_add(lo_t[:, 0:nv], p_l[:, 0:nv], ob_t[:, 0:nv])
                nc.sync.dma_start(logits_d.ap()[:, off:off + nv], lo_t[:, 0:nv])

    nc.compile()
    return nc


def _get_nc():
    if "nc" not in _CACHE:
        _CACHE["nc"] = _build()
    return _CACHE["nc"]


def _make_in_maps(inputs):
    return _shard_inputs(**inputs)


def _shard_inputs(word_input, last_hidden, encoder_hiddens, emb, w_ih, w_hh,
                  b_ih, b_hh, attn_w, attn_b, out_w, out_b):
    word_input = np.asarray(word_input)
    emb = np.asarray(emb, dtype=np.float32)
    idx = int(word_input.ravel()[0])
    x = np.ascontiguousarray(emb[idx])                       # [H]
    hp = np.asarray(last_hidden, dtype=np.float32).reshape(H)
    enc = np.asarray(encoder_hiddens, dtype=np.float32).reshape(S, H)
    w_ih = np.asarray(w_ih, dtype=np.float32)
    w_hh = np.asarray(w_hh, dtype=np.float32)
    b_ih = np.asarray(b_ih, dtype=np.float32)
    b_hh = np.asarray(b_hh, dtype=np.float32)
    attn_w = np.ascontiguousarray(np.asarray(attn_w, dtype=np.float32))
    out_w = np.asarray(out_w, dtype=np.float32)
    out_b = np.asarray(out_b, dtype=np.float32)

    x_pm = np.ascontiguousarray(x.reshape(8, 128).T)
    hp_pm = np.ascontiguousarray(hp.reshape(8, 128).T)
    ident = np.eye(128, dtype=np.float32)
    b_sum = b_ih + b_hh

    out_w_pad = np.zeros((VPAD, 2 * H), dtype=np.float32)
    out_w_pad[:V] = out_w
    out_b_pad = np.zeros(VPAD, dtype=np.float32)
    out_b_pad[:V] = out_b

    in_maps = []
    for c in range(NCORES):
        rows = np.concatenate([np.arange(c * HC, (c + 1) * HC) + q * H
                               for q in range(3)])
        enc_sl = enc[c * SC:(c + 1) * SC]
        in_maps.append({
            "x_pm": x_pm,
            "hp_pm": hp_pm,
            "hp_sl": np.ascontiguousarray(hp[c * HC:(c + 1) * HC]).reshape(1, HC),
            "wih_t": np.ascontiguousarray(w_ih[rows].T),
            "whh_t": np.ascontiguousarray(w_hh[rows].T),
            "b_rz": np.ascontiguousarray(b_sum[rows][:2 * HC]).reshape(1, 2 * HC),
            "b_in": np.ascontiguousarray(b_ih[rows][2 * HC:]).reshape(1, HC),
            "b_hn": np.ascontiguousarray(b_hh[rows][2 * HC:]).reshape(1, HC),
            "aw": attn_w,
            "enc_t": np.ascontiguousarray(enc_sl.T),
            "enc_n": np.ascontiguousarray(enc_sl),
            "wt": np.ascontiguousarray(out_w_pad[c * VC:(c + 1) * VC].T),
            "ob": np.ascontiguousarray(out_b_pad[c * VC:(c + 1) * VC]).reshape(1, VC),
            "ident": ident,
        })
    return in_maps


def kernel(word_input, last_hidden, encoder_hiddens, emb, w_ih, w_hh,
           b_ih, b_hh, attn_w, attn_b, out_w, out_b):
    nc = _get_nc()
    in_maps = _shard_inputs(word_input, last_hidden, encoder_hiddens, emb,
                            w_ih, w_hh, b_ih, b_hh, attn_w, attn_b,
                            out_w, out_b)

    res = bass_utils.run_bass_kernel_spmd(nc, in_maps, core_ids=list(range(NCORES)))

    logits = np.concatenate([res.results[c]["logits"][0] for c in range(NCORES)])
    logits = logits[:V].reshape(1, V)
    hidden = res.results[0]["hidden"].reshape(1, 1, H)
    attnw = res.results[0]["attnw"].reshape(1, 1, S)
    return logits, hidden, attnw
